# revision 22
# baseline (speedup 1.0000x reference)
"""GRU cell kernel for Trainium2, data-parallel across 8 NeuronCores.

Reference computation (per batch row):
    concat = [h_prev, x]                       # [B, 2048]
    z = sigmoid(concat @ W_z.T + b_z)          # [B, 1024]
    r = sigmoid(concat @ W_r.T + b_r)
    h_tilde = tanh([r*h_prev, x] @ W_h.T + b_h)
    h = (1-z)*h_prev + z*h_tilde

Sharding: batch dim (8192) split 1024/core; weights replicated.

Matmuls run in fp8 e4m3 with MatmulPerfMode.DoubleRow: each instruction
consumes TWO 128-deep contraction chunks ([128, 2, free] operands), at
0.5 PE cycles per moving column -> 4x bf16 MAC throughput. Weights are
prescaled by 256 on the host so they sit in e4m3's normal range
(|W| ~ 0.022 would otherwise be denormal); the 1/256 descale folds into
the ACT activation's `scale` operand.  Elementwise runs in bf16 (2x DVE
rate), h_prev is kept as a bf16 copy for the state-carry path, and
c = (1-z)*h_prev is precomputed during the z-stage so the h-stage tail
only needs two DVE ops per tile.  Expected rel err ~1.7e-2 (fp8 operand
quantization, dominated by the h/z gates); `fp8wh`/`fp8whz` variants add
a second weight-residual matmul pass on W_h (and W_z) to cut it to
~1.5e-2/~1.3e-2 at +13.7us PE each.

All DRAM layouts are partition-major so every DMA is contiguous per
partition (>=2KB descriptors): the baseline trace showed the 17us ramp
was descriptor/latency-bound, not bandwidth-bound.
"""

import numpy as np

import concourse.bacc as bacc
import concourse.bass as bass
import concourse.mybir as mybir
import concourse.tile as tile
from concourse import bass_utils

P = 128
B = 8192
I = 1024
H = 1024
K = I + H            # 2048 contraction
NCORES = 8
BS = B // NCORES     # 1024 batch rows per core
MT = H // P          # 8 m-tiles (hidden units)
KT = K // P          # 16 k-chunks
KP = KT // 2         # 8 DoubleRow pairs (0-3 h-part, 4-7 x-part)
NF = 512             # matmul moving free dim (one PSUM bank of fp32)
NT = BS // NF        # 2 n-tiles per core
WSCALE = 256.0       # host-side weight prescale for e4m3 range

F32 = mybir.dt.float32
BF16 = mybir.dt.bfloat16
F8 = mybir.dt.float8e4

AF = mybir.ActivationFunctionType
DR = mybir.MatmulPerfMode.DoubleRow


def build_kernel(variant: str = "fp8"):
    """Build the per-core Bass kernel. Returns compiled nc."""
    wlo_h = variant in ("fp8wh", "fp8whz")
    wlo_z = variant == "fp8whz"
    nc = bacc.Bacc("TRN2", target_bir_lowering=False, debug=False)

    # DRAM I/O (per-core shapes), all partition-major.
    x8d = nc.dram_tensor("x8", [P, NT, KP, NF], F8, kind="ExternalInput").ap()
    h8d = nc.dram_tensor("h8", [P, NT, KP, NF], F8, kind="ExternalInput").ap()
    hpbd = nc.dram_tensor("hpb", [P, NT, MT, NF], BF16,
                          kind="ExternalInput").ap()
    Wrd = nc.dram_tensor("Wr", [P, MT, KT, P], F8, kind="ExternalInput").ap()
    Wzd = nc.dram_tensor("Wz", [P, MT, KT, P], F8, kind="ExternalInput").ap()
    Whd = nc.dram_tensor("Wh", [P, MT, KT, P], F8, kind="ExternalInput").ap()
    Whl = (nc.dram_tensor("Whl", [P, MT, KT, P], F8, kind="ExternalInput").ap()
           if wlo_h else None)
    Wzl = (nc.dram_tensor("Wzl", [P, MT, KT, P], F8, kind="ExternalInput").ap()
           if wlo_z else None)
    brd = nc.dram_tensor("br", [P, MT], F32, kind="ExternalInput").ap()
    bzd = nc.dram_tensor("bz", [P, MT], F32, kind="ExternalInput").ap()
    bhd = nc.dram_tensor("bh", [P, MT], F32, kind="ExternalInput").ap()
    out = nc.dram_tensor("out", [H, BS], BF16, kind="ExternalOutput").ap()

    with tile.TileContext(nc) as tc:
        with (
            tc.tile_pool(name="acts", bufs=1) as acts,
            tc.tile_pool(name="gates", bufs=1) as gates,
            tc.tile_pool(name="opool", bufs=8) as opool,
            tc.tile_pool(name="ppool", bufs=8, space="PSUM") as ppool,
        ):
            # Persistent SBUF state
            x8_sb = acts.tile([P, NT, KP, NF], F8)
            h8_sb = acts.tile([P, NT, KP, NF], F8)
            hpb_sb = acts.tile([P, NT, MT, NF], BF16)
            wr_sb = acts.tile([P, MT, KT, P], F8)
            wz_sb = acts.tile([P, MT, KT, P], F8)
            wh_sb = acts.tile([P, MT, KT, P], F8)
            whl_sb = acts.tile([P, MT, KT, P], F8, name="whl") if wlo_h else None
            wzl_sb = acts.tile([P, MT, KT, P], F8, name="wzl") if wlo_z else None
            br_sb = acts.tile([P, MT], F32)
            bz_sb = acts.tile([P, MT], F32)
            bh_sb = acts.tile([P, MT], F32)
            r_sb = gates.tile([P, NT, MT, NF], BF16)
            z_sb = gates.tile([P, NT, MT, NF], BF16)
            c_sb = gates.tile([P, NT, MT, NF], BF16)
            rh_sb = gates.tile([P, NT, KP, NF], F8)

            # PE pstate pre-warm scratch, zeroed on the (otherwise idle)
            # vector queue so the dummy matmuls can start ~4us in.
            scr_w = acts.tile([P, 2, P], F8, name="scr_w")
            scr_m = acts.tile([P, 2, NF], F8, name="scr_m")
            nc.vector.memset(scr_w[:], 0)
            nc.vector.memset(scr_m[:], 0)

            # DMA schedule.  A ring drains FIFO, so the sync ring is a strict
            # priority queue: every transfer the PE consumes, in consumption
            # order.  Everything rides the ONE sync DGE ring -- each extra
            # ring adds ~1.5-3us of queue init + end-of-kernel drain.
            nc.sync.dma_start(wr_sb[:, 0, 0:8], Wrd[:, 0, 0:8])
            nc.sync.dma_start(h8_sb[:, 0, 0:4], h8d[:, 0, 0:4])
            nc.sync.dma_start(wr_sb[:, 0, 8:16], Wrd[:, 0, 8:16])
            nc.sync.dma_start(x8_sb[:, 0, 0:4], x8d[:, 0, 0:4])
            nc.sync.dma_start(wr_sb[:, 1], Wrd[:, 1])
            nc.sync.dma_start(h8_sb[:, 0, 4:8], h8d[:, 0, 4:8])
            nc.sync.dma_start(x8_sb[:, 0, 4:8], x8d[:, 0, 4:8])
            nc.sync.dma_start(wr_sb[:, 2], Wrd[:, 2])
            nc.sync.dma_start(wr_sb[:, 3], Wrd[:, 3])
            nc.sync.dma_start(br_sb[:], brd)
            nc.sync.dma_start(bz_sb[:], bzd)
            nc.sync.dma_start(bh_sb[:], bhd)
            nc.sync.dma_start(wr_sb[:, 4:8], Wrd[:, 4:8])
            nc.sync.dma_start(h8_sb[:, 1], h8d[:, 1])
            nc.sync.dma_start(x8_sb[:, 1], x8d[:, 1])
            nc.sync.dma_start(wz_sb[:, 0:4], Wzd[:, 0:4])
            nc.sync.dma_start(wz_sb[:, 4:8], Wzd[:, 4:8])
            if wlo_z:
                nc.sync.dma_start(wzl_sb[:], Wzl)
            nc.sync.dma_start(hpb_sb[:, 0], hpbd[:, 0])
            nc.sync.dma_start(wh_sb[:, 0:4], Whd[:, 0:4])
            nc.sync.dma_start(wh_sb[:, 4:8], Whd[:, 4:8])
            if wlo_h:
                nc.sync.dma_start(whl_sb[:], Whl)
            nc.sync.dma_start(hpb_sb[:, 1], hpbd[:, 1])

            # Pre-warm both ACT tables during the DMA fill so no real
            # activation pays the table-load latency.
            warm = acts.tile([P, 1], F32)
            warm2 = acts.tile([P, 1], F32, name="warm2")
            nc.scalar.activation(warm[:], warm[:], AF.Sigmoid)
            nc.scalar.activation(warm2[:], warm2[:], AF.Tanh)

            # Pre-warm the PE pstate: dummy DoubleRow matmuls on the zeroed
            # scratch keep the PE clocked up through the DMA ramp so the real
            # matmuls start at full speed instead of spending their first
            # ~3us at the mid pstate.  Result is discarded.
            scr_ps = ppool.tile([P, NF], F32, tag="ps", name="scr_ps")
            for _ in range(10):
                nc.tensor.matmul(scr_ps, scr_w[:], scr_m[:],
                                 start=True, stop=True, perf_mode=DR)

            def moving(stage, n, kp, base, width):
                """DoubleRow moving operand [128, 2, width] for pair kp."""
                if kp < KP // 2:
                    src = rh_sb if stage == "h" else h8_sb
                    return src[:, n, 2 * kp:2 * kp + 2, base:base + width]
                kx = 2 * (kp - KP // 2)
                return x8_sb[:, n, kx:kx + 2, base:base + width]

            def mm_group(stage, w_sb, wl_sb, mt, n, ps, base, width):
                nmm = KP if wl_sb is None else 2 * KP
                i = 0
                for w in ([w_sb] if wl_sb is None else [w_sb, wl_sb]):
                    for kp in range(KP):
                        nc.tensor.matmul(
                            ps, w[:, mt, 2 * kp:2 * kp + 2, :],
                            moving(stage, n, kp, base, width),
                            start=(i == 0), stop=(i == nmm - 1), perf_mode=DR)
                        i += 1

            def finish(stage, mt, n, ps, base, width):
                ns = slice(base, base + width)
                if stage == "r":
                    nc.scalar.activation(
                        r_sb[:, n, mt, ns], ps, AF.Sigmoid,
                        bias=br_sb[:, mt:mt + 1], scale=1.0 / WSCALE)
                    nc.vector.tensor_mul(
                        rh_sb[:, n, mt, ns], r_sb[:, n, mt, ns],
                        hpb_sb[:, n, mt, ns])
                elif stage == "z":
                    nc.scalar.activation(
                        z_sb[:, n, mt, ns], ps, AF.Sigmoid,
                        bias=bz_sb[:, mt:mt + 1], scale=1.0 / WSCALE)
                    t1 = opool.tile([P, width], BF16, tag="t1")
                    nc.vector.tensor_mul(
                        t1, z_sb[:, n, mt, ns], hpb_sb[:, n, mt, ns])
                    nc.vector.tensor_sub(
                        c_sb[:, n, mt, ns], hpb_sb[:, n, mt, ns], t1)
                else:
                    ht = opool.tile([P, width], BF16, tag="ht")
                    nc.scalar.activation(
                        ht, ps, AF.Tanh,
                        bias=bh_sb[:, mt:mt + 1], scale=1.0 / WSCALE)
                    t2 = opool.tile([P, width], BF16, tag="t2")
                    nc.vector.tensor_mul(t2, z_sb[:, n, mt, ns], ht)
                    ho = opool.tile([P, width], BF16, tag="ho")
                    nc.vector.tensor_add(ho, c_sb[:, n, mt, ns], t2)
                    nc.sync.dma_start(
                        out[mt * P:(mt + 1) * P,
                            n * NF + base:n * NF + base + width], ho)

            def gate(stage, w_sb, wl_sb):
                plan = ([(mt, 0) for mt in range(MT)]
                        + [(mt, 1) for mt in range(MT)])
                for mt, n in plan:
                    # Split the very last group so its ACT+DVE+store chain
                    # pipelines instead of sitting exposed after the PE ends.
                    last = stage == "h" and mt == MT - 1 and n == NT - 1
                    nsub = 2 if last else 1
                    width = NF // nsub
                    for si in range(nsub):
                        base = si * width
                        ps = ppool.tile([P, width], F32, tag="ps",
                                        name=f"ps{stage}{mt}_{n}_{si}")
                        mm_group(stage, w_sb, wl_sb, mt, n, ps, base, width)
                        finish(stage, mt, n, ps, base, width)

            gate("r", wr_sb, None)
            gate("z", wz_sb, wzl_sb)
            gate("h", wh_sb, whl_sb)

    nc.compile()
    return nc


def _prep_inputs(x, h_prev, W_z, b_z, W_r, b_r, W_h, b_h, variant="fp8"):
    """Host-side relayout: partition-major, fp8/bf16 quantization."""
    import ml_dtypes
    E4 = ml_dtypes.float8_e4m3
    BF = ml_dtypes.bfloat16
    wlo_h = variant in ("fp8wh", "fp8whz")
    wlo_z = variant == "fp8whz"

    def prep_w(W):
        # [p, mt, k, m] <- W[mt*128+m, k*128+p], f32, prescaled
        W4 = W.reshape(MT, P, KT, P)          # [mt, m, k, p]
        Wt = np.ascontiguousarray(W4.transpose(3, 0, 2, 1)) * WSCALE
        Whi = Wt.astype(E4)
        lo = (Wt - Whi.astype(np.float32)).astype(E4)
        return Whi, lo

    def prep_b(b):
        return np.ascontiguousarray(b.reshape(MT, P).T)

    Wr8, Wrl = prep_w(W_r)
    Wz8, Wzl = prep_w(W_z)
    Wh8, Whl = prep_w(W_h)
    shared = {
        "Wr": Wr8, "Wz": Wz8, "Wh": Wh8,
        "br": prep_b(b_r), "bz": prep_b(b_z), "bh": prep_b(b_h),
    }
    if wlo_h:
        shared["Whl"] = Whl
    if wlo_z:
        shared["Wzl"] = Wzl

    def prep_act(a, dt):
        # [p, n, ko, j] <- a[n*NF+j, ko*128+p]  (a is the per-core slice)
        a4 = a.reshape(NT, NF, KP, P)          # [n, j, ko, p]
        return np.ascontiguousarray(a4.transpose(3, 0, 2, 1)).astype(dt)

    in_maps = []
    for c in range(NCORES):
        bs = slice(c * BS, (c + 1) * BS)
        m = dict(shared)
        m["x8"] = prep_act(x[bs], E4)
        m["h8"] = prep_act(h_prev[bs], E4)
        m["hpb"] = prep_act(h_prev[bs], BF)
        in_maps.append(m)
    return in_maps


def run(inputs, mm_dtype="fp8", trace=False, **run_kwargs):
    """Compile + run on 8 cores. Returns (output [B,H] f32, results)."""
    variant = mm_dtype if mm_dtype in ("fp8", "fp8wh", "fp8whz") else "fp8"
    nc = build_kernel(variant)
    in_maps = _prep_inputs(**inputs, variant=variant)
    res = bass_utils.run_bass_kernel_spmd(
        nc, in_maps, core_ids=list(range(NCORES)), trace=trace, **run_kwargs)
    outT = np.concatenate(
        [res.results[c]["out"] for c in range(NCORES)], axis=1)  # [H, B] bf16
    return np.ascontiguousarray(outT.T).astype(np.float32), res


def kernel(**inputs) -> np.ndarray:
    import time as _time
    try:
        out, _ = run(inputs)
    except Exception:
        # The axon-tunneled device occasionally reports a transient
        # "unrecoverable" state right after a crashed session; a fresh
        # attempt after a short pause recovers.
        _time.sleep(15)
        out, _ = run(inputs)
    return out


# revision 23
# speedup vs baseline: 1.0139x; 1.0139x over previous
"""GRU cell kernel for Trainium2, data-parallel across 8 NeuronCores.

Reference computation (per batch row):
    concat = [h_prev, x]                       # [B, 2048]
    z = sigmoid(concat @ W_z.T + b_z)          # [B, 1024]
    r = sigmoid(concat @ W_r.T + b_r)
    h_tilde = tanh([r*h_prev, x] @ W_h.T + b_h)
    h = (1-z)*h_prev + z*h_tilde

Sharding: batch dim (8192) split 1024/core; weights replicated.

Matmuls run in fp8 e4m3 with MatmulPerfMode.DoubleRow: each instruction
consumes TWO 128-deep contraction chunks ([128, 2, free] operands), at
0.5 PE cycles per moving column -> 4x bf16 MAC throughput. Weights are
prescaled by 256 on the host so they sit in e4m3's normal range
(|W| ~ 0.022 would otherwise be denormal); the 1/256 descale folds into
the ACT activation's `scale` operand.  Elementwise runs in bf16 (2x DVE
rate), h_prev is kept as a bf16 copy for the state-carry path, and
c = (1-z)*h_prev is precomputed during the z-stage so the h-stage tail
only needs two DVE ops per tile.  Expected rel err ~1.7e-2 (fp8 operand
quantization, dominated by the h/z gates); `fp8wh`/`fp8whz` variants add
a second weight-residual matmul pass on W_h (and W_z) to cut it to
~1.5e-2/~1.3e-2 at +13.7us PE each.

All DRAM layouts are partition-major so every DMA is contiguous per
partition (>=2KB descriptors): the baseline trace showed the 17us ramp
was descriptor/latency-bound, not bandwidth-bound.
"""

import numpy as np

import concourse.bacc as bacc
import concourse.bass as bass
import concourse.mybir as mybir
import concourse.tile as tile
from concourse import bass_utils

P = 128
B = 8192
I = 1024
H = 1024
K = I + H            # 2048 contraction
NCORES = 8
BS = B // NCORES     # 1024 batch rows per core
MT = H // P          # 8 m-tiles (hidden units)
KT = K // P          # 16 k-chunks
KP = KT // 2         # 8 DoubleRow pairs (0-3 h-part, 4-7 x-part)
NF = 512             # matmul moving free dim (one PSUM bank of fp32)
NT = BS // NF        # 2 n-tiles per core
WSCALE = 256.0       # host-side weight prescale for e4m3 range

F32 = mybir.dt.float32
BF16 = mybir.dt.bfloat16
F8 = mybir.dt.float8e4

AF = mybir.ActivationFunctionType
DR = mybir.MatmulPerfMode.DoubleRow


def build_kernel(variant: str = "fp8"):
    """Build the per-core Bass kernel. Returns compiled nc."""
    wlo_h = variant in ("fp8wh", "fp8whz")
    wlo_z = variant == "fp8whz"
    nc = bacc.Bacc("TRN2", target_bir_lowering=False, debug=False)

    # DRAM I/O (per-core shapes), all partition-major.
    x8d = nc.dram_tensor("x8", [P, NT, KP, NF], F8, kind="ExternalInput").ap()
    h8d = nc.dram_tensor("h8", [P, NT, KP, NF], F8, kind="ExternalInput").ap()
    hpbd = nc.dram_tensor("hpb", [P, NT, MT, NF], BF16,
                          kind="ExternalInput").ap()
    Wrd = nc.dram_tensor("Wr", [P, MT, KT, P], F8, kind="ExternalInput").ap()
    Wzd = nc.dram_tensor("Wz", [P, MT, KT, P], F8, kind="ExternalInput").ap()
    Whd = nc.dram_tensor("Wh", [P, MT, KT, P], F8, kind="ExternalInput").ap()
    Whl = (nc.dram_tensor("Whl", [P, MT, KT, P], F8, kind="ExternalInput").ap()
           if wlo_h else None)
    Wzl = (nc.dram_tensor("Wzl", [P, MT, KT, P], F8, kind="ExternalInput").ap()
           if wlo_z else None)
    brd = nc.dram_tensor("br", [P, MT], F32, kind="ExternalInput").ap()
    bzd = nc.dram_tensor("bz", [P, MT], F32, kind="ExternalInput").ap()
    bhd = nc.dram_tensor("bh", [P, MT], F32, kind="ExternalInput").ap()
    out = nc.dram_tensor("out", [H, BS], BF16, kind="ExternalOutput").ap()

    with tile.TileContext(nc) as tc:
        with (
            tc.tile_pool(name="acts", bufs=1) as acts,
            tc.tile_pool(name="gates", bufs=1) as gates,
            tc.tile_pool(name="opool", bufs=8) as opool,
            tc.tile_pool(name="ppool", bufs=8, space="PSUM") as ppool,
        ):
            # Persistent SBUF state
            x8_sb = acts.tile([P, NT, KP, NF], F8)
            h8_sb = acts.tile([P, NT, KP, NF], F8)
            hpb_sb = acts.tile([P, NT, MT, NF], BF16)
            wr_sb = acts.tile([P, MT, KT, P], F8)
            wz_sb = acts.tile([P, MT, KT, P], F8)
            wh_sb = acts.tile([P, MT, KT, P], F8)
            whl_sb = acts.tile([P, MT, KT, P], F8, name="whl") if wlo_h else None
            wzl_sb = acts.tile([P, MT, KT, P], F8, name="wzl") if wlo_z else None
            br_sb = acts.tile([P, MT], F32)
            bz_sb = acts.tile([P, MT], F32)
            bh_sb = acts.tile([P, MT], F32)
            r_sb = gates.tile([P, NT, MT, NF], BF16)
            z_sb = gates.tile([P, NT, MT, NF], BF16)
            c_sb = gates.tile([P, NT, MT, NF], BF16)
            rh_sb = gates.tile([P, NT, KP, NF], F8)

            # PE pstate pre-warm scratch, zeroed on the (otherwise idle)
            # vector queue so the dummy matmuls can start ~4us in.
            scr_w = acts.tile([P, 2, P], F8, name="scr_w")
            scr_m = acts.tile([P, 2, NF], F8, name="scr_m")
            nc.vector.memset(scr_w[:], 0)
            nc.vector.memset(scr_m[:], 0)

            # DMA schedule.  A ring drains FIFO, so the sync ring is a strict
            # priority queue: every transfer the PE consumes, in consumption
            # order.  Everything rides the ONE sync DGE ring -- each extra
            # ring adds ~1.5-3us of queue init + end-of-kernel drain.
            nc.sync.dma_start(wr_sb[:, 0], Wrd[:, 0])
            nc.sync.dma_start(h8_sb[:, 0, 0:4], h8d[:, 0, 0:4])
            nc.sync.dma_start(wr_sb[:, 1], Wrd[:, 1])
            nc.sync.dma_start(h8_sb[:, 0, 4:8], h8d[:, 0, 4:8])
            nc.sync.dma_start(x8_sb[:, 0], x8d[:, 0])
            nc.sync.dma_start(wr_sb[:, 2], Wrd[:, 2])
            nc.sync.dma_start(wr_sb[:, 3], Wrd[:, 3])
            nc.sync.dma_start(br_sb[:], brd)
            nc.sync.dma_start(bz_sb[:], bzd)
            nc.sync.dma_start(bh_sb[:], bhd)
            nc.sync.dma_start(wr_sb[:, 4:8], Wrd[:, 4:8])
            nc.sync.dma_start(h8_sb[:, 1], h8d[:, 1])
            nc.sync.dma_start(x8_sb[:, 1], x8d[:, 1])
            nc.sync.dma_start(wz_sb[:, 0:4], Wzd[:, 0:4])
            nc.sync.dma_start(wz_sb[:, 4:8], Wzd[:, 4:8])
            if wlo_z:
                nc.sync.dma_start(wzl_sb[:], Wzl)
            nc.sync.dma_start(hpb_sb[:, 0], hpbd[:, 0])
            nc.sync.dma_start(wh_sb[:, 0:4], Whd[:, 0:4])
            nc.sync.dma_start(wh_sb[:, 4:8], Whd[:, 4:8])
            if wlo_h:
                nc.sync.dma_start(whl_sb[:], Whl)
            nc.sync.dma_start(hpb_sb[:, 1], hpbd[:, 1])

            # Pre-warm both ACT tables during the DMA fill so no real
            # activation pays the table-load latency.
            warm = acts.tile([P, 1], F32)
            warm2 = acts.tile([P, 1], F32, name="warm2")
            nc.scalar.activation(warm[:], warm[:], AF.Sigmoid)
            nc.scalar.activation(warm2[:], warm2[:], AF.Tanh)

            # Pre-warm the PE pstate: dummy DoubleRow matmuls on the zeroed
            # scratch keep the PE clocked up through the DMA ramp so the real
            # matmuls start at full speed instead of spending their first
            # ~3us at the mid pstate.  Result is discarded.
            scr_ps = ppool.tile([P, NF], F32, tag="ps", name="scr_ps")
            for _ in range(10):
                nc.tensor.matmul(scr_ps, scr_w[:], scr_m[:],
                                 start=True, stop=True, perf_mode=DR)

            def moving(stage, n, kp, base, width):
                """DoubleRow moving operand [128, 2, width] for pair kp."""
                if kp < KP // 2:
                    src = rh_sb if stage == "h" else h8_sb
                    return src[:, n, 2 * kp:2 * kp + 2, base:base + width]
                kx = 2 * (kp - KP // 2)
                return x8_sb[:, n, kx:kx + 2, base:base + width]

            def mm_group(stage, w_sb, wl_sb, mt, n, ps, base, width):
                nmm = KP if wl_sb is None else 2 * KP
                i = 0
                for w in ([w_sb] if wl_sb is None else [w_sb, wl_sb]):
                    for kp in range(KP):
                        nc.tensor.matmul(
                            ps, w[:, mt, 2 * kp:2 * kp + 2, :],
                            moving(stage, n, kp, base, width),
                            start=(i == 0), stop=(i == nmm - 1), perf_mode=DR)
                        i += 1

            def finish(stage, mt, n, ps, base, width):
                ns = slice(base, base + width)
                if stage == "r":
                    nc.scalar.activation(
                        r_sb[:, n, mt, ns], ps, AF.Sigmoid,
                        bias=br_sb[:, mt:mt + 1], scale=1.0 / WSCALE)
                    nc.vector.tensor_mul(
                        rh_sb[:, n, mt, ns], r_sb[:, n, mt, ns],
                        hpb_sb[:, n, mt, ns])
                elif stage == "z":
                    nc.scalar.activation(
                        z_sb[:, n, mt, ns], ps, AF.Sigmoid,
                        bias=bz_sb[:, mt:mt + 1], scale=1.0 / WSCALE)
                    t1 = opool.tile([P, width], BF16, tag="t1")
                    nc.vector.tensor_mul(
                        t1, z_sb[:, n, mt, ns], hpb_sb[:, n, mt, ns])
                    nc.vector.tensor_sub(
                        c_sb[:, n, mt, ns], hpb_sb[:, n, mt, ns], t1)
                else:
                    ht = opool.tile([P, width], BF16, tag="ht")
                    nc.scalar.activation(
                        ht, ps, AF.Tanh,
                        bias=bh_sb[:, mt:mt + 1], scale=1.0 / WSCALE)
                    t2 = opool.tile([P, width], BF16, tag="t2")
                    nc.vector.tensor_mul(t2, z_sb[:, n, mt, ns], ht)
                    ho = opool.tile([P, width], BF16, tag="ho")
                    nc.vector.tensor_add(ho, c_sb[:, n, mt, ns], t2)
                    nc.sync.dma_start(
                        out[mt * P:(mt + 1) * P,
                            n * NF + base:n * NF + base + width], ho)

            def gate(stage, w_sb, wl_sb):
                if stage == "r":
                    # Ramp: k-outer across (mt0, mt1) at n=0 so the PE
                    # starts on Wr0/Wr1 + h8-n0 only (~1MB landed).
                    NG = 2
                    pss = [ppool.tile([P, NF], F32, tag="ps",
                                      name=f"psri{g}") for g in range(NG)]
                    for kp in range(KP):
                        for g in range(NG):
                            nc.tensor.matmul(
                                pss[g], w_sb[:, g, 2 * kp:2 * kp + 2, :],
                                moving(stage, 0, kp, 0, NF),
                                start=(kp == 0), stop=(kp == KP - 1),
                                perf_mode=DR)
                    for g in range(NG):
                        finish(stage, g, 0, pss[g], 0, NF)
                    plan = ([(mt, 0) for mt in range(NG, MT)]
                            + [(mt, 1) for mt in range(MT)])
                else:
                    plan = ([(mt, 0) for mt in range(MT)]
                            + [(mt, 1) for mt in range(MT)])
                for mt, n in plan:
                    # Split the very last group so its ACT+DVE+store chain
                    # pipelines instead of sitting exposed after the PE ends.
                    last = stage == "h" and mt == MT - 1 and n == NT - 1
                    nsub = 2 if last else 1
                    width = NF // nsub
                    for si in range(nsub):
                        base = si * width
                        ps = ppool.tile([P, width], F32, tag="ps",
                                        name=f"ps{stage}{mt}_{n}_{si}")
                        mm_group(stage, w_sb, wl_sb, mt, n, ps, base, width)
                        finish(stage, mt, n, ps, base, width)

            gate("r", wr_sb, None)
            gate("z", wz_sb, wzl_sb)
            gate("h", wh_sb, whl_sb)

    nc.compile()
    return nc


def _prep_inputs(x, h_prev, W_z, b_z, W_r, b_r, W_h, b_h, variant="fp8"):
    """Host-side relayout: partition-major, fp8/bf16 quantization."""
    import ml_dtypes
    E4 = ml_dtypes.float8_e4m3
    BF = ml_dtypes.bfloat16
    wlo_h = variant in ("fp8wh", "fp8whz")
    wlo_z = variant == "fp8whz"

    def prep_w(W):
        # [p, mt, k, m] <- W[mt*128+m, k*128+p], f32, prescaled
        W4 = W.reshape(MT, P, KT, P)          # [mt, m, k, p]
        Wt = np.ascontiguousarray(W4.transpose(3, 0, 2, 1)) * WSCALE
        Whi = Wt.astype(E4)
        lo = (Wt - Whi.astype(np.float32)).astype(E4)
        return Whi, lo

    def prep_b(b):
        return np.ascontiguousarray(b.reshape(MT, P).T)

    Wr8, Wrl = prep_w(W_r)
    Wz8, Wzl = prep_w(W_z)
    Wh8, Whl = prep_w(W_h)
    shared = {
        "Wr": Wr8, "Wz": Wz8, "Wh": Wh8,
        "br": prep_b(b_r), "bz": prep_b(b_z), "bh": prep_b(b_h),
    }
    if wlo_h:
        shared["Whl"] = Whl
    if wlo_z:
        shared["Wzl"] = Wzl

    def prep_act(a, dt):
        # [p, n, ko, j] <- a[n*NF+j, ko*128+p]  (a is the per-core slice)
        a4 = a.reshape(NT, NF, KP, P)          # [n, j, ko, p]
        return np.ascontiguousarray(a4.transpose(3, 0, 2, 1)).astype(dt)

    in_maps = []
    for c in range(NCORES):
        bs = slice(c * BS, (c + 1) * BS)
        m = dict(shared)
        m["x8"] = prep_act(x[bs], E4)
        m["h8"] = prep_act(h_prev[bs], E4)
        m["hpb"] = prep_act(h_prev[bs], BF)
        in_maps.append(m)
    return in_maps


def run(inputs, mm_dtype="fp8", trace=False, **run_kwargs):
    """Compile + run on 8 cores. Returns (output [B,H] f32, results)."""
    variant = mm_dtype if mm_dtype in ("fp8", "fp8wh", "fp8whz") else "fp8"
    nc = build_kernel(variant)
    in_maps = _prep_inputs(**inputs, variant=variant)
    res = bass_utils.run_bass_kernel_spmd(
        nc, in_maps, core_ids=list(range(NCORES)), trace=trace, **run_kwargs)
    outT = np.concatenate(
        [res.results[c]["out"] for c in range(NCORES)], axis=1)  # [H, B] bf16
    return np.ascontiguousarray(outT.T).astype(np.float32), res


def kernel(**inputs) -> np.ndarray:
    import time as _time
    try:
        out, _ = run(inputs)
    except Exception:
        # The axon-tunneled device occasionally reports a transient
        # "unrecoverable" state right after a crashed session; a fresh
        # attempt after a short pause recovers.
        _time.sleep(15)
        out, _ = run(inputs)
    return out


# revision 25
# speedup vs baseline: 1.0228x; 1.0087x over previous
"""GRU cell kernel for Trainium2, data-parallel across 8 NeuronCores.

Reference computation (per batch row):
    concat = [h_prev, x]                       # [B, 2048]
    z = sigmoid(concat @ W_z.T + b_z)          # [B, 1024]
    r = sigmoid(concat @ W_r.T + b_r)
    h_tilde = tanh([r*h_prev, x] @ W_h.T + b_h)
    h = (1-z)*h_prev + z*h_tilde

Sharding: batch dim (8192) split 1024/core; weights replicated.

Matmuls run in fp8 e4m3 with MatmulPerfMode.DoubleRow: each instruction
consumes TWO 128-deep contraction chunks ([128, 2, free] operands).
Measured on TRN2 a DoubleRow matmul takes the same 216ns as a bf16 one
while doing 2x the MACs -> 2x bf16 throughput (157 TF/s; the CoreSim
cost model's 0.5 cycles/row = 4x is NOT what the hardware does), so the
3-gate PE floor is ~83us/core.  Weights are prescaled by 256 on the
host so they sit in e4m3's normal range (|W| ~ 0.022 would otherwise be
denormal); the 1/256 descale folds into the ACT activation's `scale`
operand.  Elementwise runs in bf16 (2x DVE rate), h_prev is kept as a
bf16 copy for the state-carry path, and c = (1-z)*h_prev is precomputed
during the z-stage so the h-stage tail only needs two DVE ops per tile.
Measured rel err 1.76e-2 (fp8 operand quantization, dominated by the
h/z gates; numpy-sim-exact); `fp8wh`/`fp8whz` variants add a
weight-residual matmul pass on W_h (and W_z) to cut it to
~1.5e-2/~1.3e-2 at +27us PE each.

Schedule notes (measured, HW exec ~103-104us vs 195us bf16 baseline):
- All input DMAs ride ONE sync-ring FIFO in exact PE-consumption order;
  a second DGE ring costs ~1.5-3us of extra queue init + drain, and any
  concurrent ring steals pool bandwidth from the critical chain.
- A DMA becomes visible ~6us after its descriptor-gen (~0.6us each,
  serial per sequencer), and compute engines start ~3.4us (preamble) to
  ~6.5us (ACT/DVE first op) in, so the first real matmul cannot land
  before ~11-12us; dummy DoubleRow matmuls on zeroed scratch bridge the
  gap and absorb the PE pstate ramp (~2x slow for the first ~3us).
- All DRAM layouts are partition-major so every transfer is contiguous
  per partition.
"""

import numpy as np

import concourse.bacc as bacc
import concourse.bass as bass
import concourse.mybir as mybir
import concourse.tile as tile
from concourse import bass_utils

P = 128
B = 8192
I = 1024
H = 1024
K = I + H            # 2048 contraction
NCORES = 8
BS = B // NCORES     # 1024 batch rows per core
MT = H // P          # 8 m-tiles (hidden units)
KT = K // P          # 16 k-chunks
KP = KT // 2         # 8 DoubleRow pairs (0-3 h-part, 4-7 x-part)
NF = 512             # matmul moving free dim (one PSUM bank of fp32)
NT = BS // NF        # 2 n-tiles per core
WSCALE = 256.0       # host-side weight prescale for e4m3 range

F32 = mybir.dt.float32
BF16 = mybir.dt.bfloat16
F8 = mybir.dt.float8e4

AF = mybir.ActivationFunctionType
DR = mybir.MatmulPerfMode.DoubleRow


def build_kernel(variant: str = "fp8"):
    """Build the per-core Bass kernel. Returns compiled nc."""
    wlo_h = variant in ("fp8wh", "fp8whz")
    wlo_z = variant == "fp8whz"
    nc = bacc.Bacc("TRN2", target_bir_lowering=False, debug=False)

    # DRAM I/O (per-core shapes), all partition-major.
    x8d = nc.dram_tensor("x8", [P, NT, KP, NF], F8, kind="ExternalInput").ap()
    h8d = nc.dram_tensor("h8", [P, NT, KP, NF], F8, kind="ExternalInput").ap()
    hpbd = nc.dram_tensor("hpb", [P, NT, MT, NF], BF16,
                          kind="ExternalInput").ap()
    Wrd = nc.dram_tensor("Wr", [P, MT, KT, P], F8, kind="ExternalInput").ap()
    Wzd = nc.dram_tensor("Wz", [P, MT, KT, P], F8, kind="ExternalInput").ap()
    Whd = nc.dram_tensor("Wh", [P, MT, KT, P], F8, kind="ExternalInput").ap()
    Whl = (nc.dram_tensor("Whl", [P, MT, KT, P], F8, kind="ExternalInput").ap()
           if wlo_h else None)
    Wzl = (nc.dram_tensor("Wzl", [P, MT, KT, P], F8, kind="ExternalInput").ap()
           if wlo_z else None)
    brd = nc.dram_tensor("br", [P, MT], F32, kind="ExternalInput").ap()
    bzd = nc.dram_tensor("bz", [P, MT], F32, kind="ExternalInput").ap()
    bhd = nc.dram_tensor("bh", [P, MT], F32, kind="ExternalInput").ap()
    out = nc.dram_tensor("out", [H, BS], BF16, kind="ExternalOutput").ap()

    with tile.TileContext(nc) as tc:
        with (
            tc.tile_pool(name="acts", bufs=1) as acts,
            tc.tile_pool(name="gates", bufs=1) as gates,
            tc.tile_pool(name="opool", bufs=8) as opool,
            tc.tile_pool(name="ppool", bufs=8, space="PSUM") as ppool,
        ):
            # Persistent SBUF state
            x8_sb = acts.tile([P, NT, KP, NF], F8)
            h8_sb = acts.tile([P, NT, KP, NF], F8)
            hpb_sb = acts.tile([P, NT, MT, NF], BF16)
            wr_sb = acts.tile([P, MT, KT, P], F8)
            wz_sb = acts.tile([P, MT, KT, P], F8)
            wh_sb = acts.tile([P, MT, KT, P], F8)
            whl_sb = acts.tile([P, MT, KT, P], F8, name="whl") if wlo_h else None
            wzl_sb = acts.tile([P, MT, KT, P], F8, name="wzl") if wlo_z else None
            br_sb = acts.tile([P, MT], F32)
            bz_sb = acts.tile([P, MT], F32)
            bh_sb = acts.tile([P, MT], F32)
            r_sb = gates.tile([P, NT, MT, NF], BF16)
            z_sb = gates.tile([P, NT, MT, NF], BF16)
            c_sb = gates.tile([P, NT, MT, NF], BF16)
            rh_sb = gates.tile([P, NT, KP, NF], F8)

            # PE pstate pre-warm scratch, zeroed on the (otherwise idle)
            # vector queue so the dummy matmuls can start as soon as the
            # engines come up (~8us in).
            scr_w = acts.tile([P, 2, P], F8, name="scr_w")
            scr_m = acts.tile([P, 2, NF], F8, name="scr_m")
            nc.vector.memset(scr_w[:], 0)
            nc.vector.memset(scr_m[:], 0)

            # DMA schedule.  A ring drains FIFO, so the sync ring is a strict
            # priority queue: every transfer the PE consumes, in consumption
            # order.  Everything rides the ONE sync DGE ring -- each extra
            # ring adds ~1.5-3us of queue init + end-of-kernel drain.
            nc.sync.dma_start(wr_sb[:, 0], Wrd[:, 0])
            nc.sync.dma_start(h8_sb[:, 0, 0:4], h8d[:, 0, 0:4])
            nc.sync.dma_start(wr_sb[:, 1], Wrd[:, 1])
            nc.sync.dma_start(h8_sb[:, 0, 4:8], h8d[:, 0, 4:8])
            nc.sync.dma_start(x8_sb[:, 0], x8d[:, 0])
            nc.sync.dma_start(wr_sb[:, 2], Wrd[:, 2])
            nc.sync.dma_start(wr_sb[:, 3], Wrd[:, 3])
            nc.sync.dma_start(br_sb[:], brd)
            nc.sync.dma_start(bz_sb[:], bzd)
            nc.sync.dma_start(bh_sb[:], bhd)
            nc.sync.dma_start(wr_sb[:, 4:8], Wrd[:, 4:8])
            nc.sync.dma_start(h8_sb[:, 1], h8d[:, 1])
            nc.sync.dma_start(x8_sb[:, 1], x8d[:, 1])
            nc.sync.dma_start(wz_sb[:, 0:4], Wzd[:, 0:4])
            nc.sync.dma_start(wz_sb[:, 4:8], Wzd[:, 4:8])
            if wlo_z:
                nc.sync.dma_start(wzl_sb[:], Wzl)
            nc.sync.dma_start(hpb_sb[:, 0], hpbd[:, 0])
            nc.sync.dma_start(wh_sb[:, 0:4], Whd[:, 0:4])
            nc.sync.dma_start(wh_sb[:, 4:8], Whd[:, 4:8])
            if wlo_h:
                nc.sync.dma_start(whl_sb[:], Whl)
            nc.sync.dma_start(hpb_sb[:, 1], hpbd[:, 1])

            # Pre-warm both ACT tables during the DMA fill so no real
            # activation pays the table-load latency.
            warm = acts.tile([P, 1], F32)
            warm2 = acts.tile([P, 1], F32, name="warm2")
            nc.scalar.activation(warm[:], warm[:], AF.Sigmoid)
            nc.scalar.activation(warm2[:], warm2[:], AF.Tanh)

            # Pre-warm the PE pstate: dummy DoubleRow matmuls on the zeroed
            # scratch keep the PE clocked up through the DMA ramp so the real
            # matmuls start at full speed instead of spending their first
            # ~3us at the mid pstate.  Result is discarded.
            scr_ps = ppool.tile([P, NF], F32, tag="ps", name="scr_ps")
            for _ in range(10):
                nc.tensor.matmul(scr_ps, scr_w[:], scr_m[:],
                                 start=True, stop=True, perf_mode=DR)

            def moving(stage, n, kp, base, width):
                """DoubleRow moving operand [128, 2, width] for pair kp."""
                if kp < KP // 2:
                    src = rh_sb if stage == "h" else h8_sb
                    return src[:, n, 2 * kp:2 * kp + 2, base:base + width]
                kx = 2 * (kp - KP // 2)
                return x8_sb[:, n, kx:kx + 2, base:base + width]

            def mm_group(stage, w_sb, wl_sb, mt, n, ps, base, width):
                nmm = KP if wl_sb is None else 2 * KP
                i = 0
                for w in ([w_sb] if wl_sb is None else [w_sb, wl_sb]):
                    for kp in range(KP):
                        nc.tensor.matmul(
                            ps, w[:, mt, 2 * kp:2 * kp + 2, :],
                            moving(stage, n, kp, base, width),
                            start=(i == 0), stop=(i == nmm - 1), perf_mode=DR)
                        i += 1

            def finish(stage, mt, n, ps, base, width):
                ns = slice(base, base + width)
                if stage == "r":
                    nc.scalar.activation(
                        r_sb[:, n, mt, ns], ps, AF.Sigmoid,
                        bias=br_sb[:, mt:mt + 1], scale=1.0 / WSCALE)
                    nc.vector.tensor_mul(
                        rh_sb[:, n, mt, ns], r_sb[:, n, mt, ns],
                        hpb_sb[:, n, mt, ns])
                elif stage == "z":
                    nc.scalar.activation(
                        z_sb[:, n, mt, ns], ps, AF.Sigmoid,
                        bias=bz_sb[:, mt:mt + 1], scale=1.0 / WSCALE)
                    t1 = opool.tile([P, width], BF16, tag="t1")
                    nc.vector.tensor_mul(
                        t1, z_sb[:, n, mt, ns], hpb_sb[:, n, mt, ns])
                    nc.vector.tensor_sub(
                        c_sb[:, n, mt, ns], hpb_sb[:, n, mt, ns], t1)
                else:
                    ht = opool.tile([P, width], BF16, tag="ht")
                    nc.scalar.activation(
                        ht, ps, AF.Tanh,
                        bias=bh_sb[:, mt:mt + 1], scale=1.0 / WSCALE)
                    t2 = opool.tile([P, width], BF16, tag="t2")
                    nc.vector.tensor_mul(t2, z_sb[:, n, mt, ns], ht)
                    ho = opool.tile([P, width], BF16, tag="ho")
                    nc.vector.tensor_add(ho, c_sb[:, n, mt, ns], t2)
                    nc.sync.dma_start(
                        out[mt * P:(mt + 1) * P,
                            n * NF + base:n * NF + base + width], ho)

            def gate(stage, w_sb, wl_sb):
                if stage == "r":
                    # Ramp: k-outer across (mt0, mt1) at n=0 so the PE
                    # starts on Wr0/Wr1 + h8-n0 only (~1MB landed).
                    NG = 2
                    pss = [ppool.tile([P, NF], F32, tag="ps",
                                      name=f"psri{g}") for g in range(NG)]
                    for kp in range(KP):
                        for g in range(NG):
                            nc.tensor.matmul(
                                pss[g], w_sb[:, g, 2 * kp:2 * kp + 2, :],
                                moving(stage, 0, kp, 0, NF),
                                start=(kp == 0), stop=(kp == KP - 1),
                                perf_mode=DR)
                    for g in range(NG):
                        finish(stage, g, 0, pss[g], 0, NF)
                    plan = ([(mt, 0) for mt in range(NG, MT)]
                            + [(mt, 1) for mt in range(MT)])
                else:
                    plan = ([(mt, 0) for mt in range(MT)]
                            + [(mt, 1) for mt in range(MT)])
                for mt, n in plan:
                    # Split the very last group so its ACT+DVE+store chain
                    # pipelines instead of sitting exposed after the PE ends.
                    last = stage == "h" and mt == MT - 1 and n == NT - 1
                    nsub = 2 if last else 1
                    width = NF // nsub
                    for si in range(nsub):
                        base = si * width
                        ps = ppool.tile([P, width], F32, tag="ps",
                                        name=f"ps{stage}{mt}_{n}_{si}")
                        mm_group(stage, w_sb, wl_sb, mt, n, ps, base, width)
                        finish(stage, mt, n, ps, base, width)

            gate("r", wr_sb, None)
            gate("z", wz_sb, wzl_sb)
            gate("h", wh_sb, whl_sb)

    nc.compile()
    return nc


def _prep_inputs(x, h_prev, W_z, b_z, W_r, b_r, W_h, b_h, variant="fp8"):
    """Host-side relayout: partition-major, fp8/bf16 quantization."""
    import ml_dtypes
    E4 = ml_dtypes.float8_e4m3
    BF = ml_dtypes.bfloat16
    wlo_h = variant in ("fp8wh", "fp8whz")
    wlo_z = variant == "fp8whz"

    def prep_w(W):
        # [p, mt, k, m] <- W[mt*128+m, k*128+p], f32, prescaled
        W4 = W.reshape(MT, P, KT, P)          # [mt, m, k, p]
        Wt = np.ascontiguousarray(W4.transpose(3, 0, 2, 1)) * WSCALE
        Whi = Wt.astype(E4)
        lo = (Wt - Whi.astype(np.float32)).astype(E4)
        return Whi, lo

    def prep_b(b):
        return np.ascontiguousarray(b.reshape(MT, P).T)

    Wr8, Wrl = prep_w(W_r)
    Wz8, Wzl = prep_w(W_z)
    Wh8, Whl = prep_w(W_h)
    shared = {
        "Wr": Wr8, "Wz": Wz8, "Wh": Wh8,
        "br": prep_b(b_r), "bz": prep_b(b_z), "bh": prep_b(b_h),
    }
    if wlo_h:
        shared["Whl"] = Whl
    if wlo_z:
        shared["Wzl"] = Wzl

    def prep_act(a, dt):
        # [p, n, ko, j] <- a[n*NF+j, ko*128+p]  (a is the per-core slice)
        a4 = a.reshape(NT, NF, KP, P)          # [n, j, ko, p]
        return np.ascontiguousarray(a4.transpose(3, 0, 2, 1)).astype(dt)

    in_maps = []
    for c in range(NCORES):
        bs = slice(c * BS, (c + 1) * BS)
        m = dict(shared)
        m["x8"] = prep_act(x[bs], E4)
        m["h8"] = prep_act(h_prev[bs], E4)
        m["hpb"] = prep_act(h_prev[bs], BF)
        in_maps.append(m)
    return in_maps


def run(inputs, mm_dtype="fp8", trace=False, **run_kwargs):
    """Compile + run on 8 cores. Returns (output [B,H] f32, results)."""
    variant = mm_dtype if mm_dtype in ("fp8", "fp8wh", "fp8whz") else "fp8"
    nc = build_kernel(variant)
    in_maps = _prep_inputs(**inputs, variant=variant)
    res = bass_utils.run_bass_kernel_spmd(
        nc, in_maps, core_ids=list(range(NCORES)), trace=trace, **run_kwargs)
    outT = np.concatenate(
        [res.results[c]["out"] for c in range(NCORES)], axis=1)  # [H, B] bf16
    return np.ascontiguousarray(outT.T).astype(np.float32), res


def kernel(**inputs) -> np.ndarray:
    import time as _time
    try:
        out, _ = run(inputs)
    except Exception:
        # The axon-tunneled device occasionally reports a transient
        # "unrecoverable" state right after a crashed session; a fresh
        # attempt after a short pause recovers.
        _time.sleep(15)
        out, _ = run(inputs)
    return out
